# revision 14
# baseline (speedup 1.0000x reference)
"""Trainium2 Bass kernel for nn_BackwardStep_38749194944853.

Batched ADMM QP solve (OSQP-style), N=1024 independent QPs of dim nx=128 with
mi=128 inequality + me=32 doubled equality constraints, 100 fixed iterations.

Strategy (pure data-parallel over 8 cores, 128 QPs per core):
  Phase A (per element, TensorE-heavy):
    K = Q + (1+sigma) I + rho (Ai'Ai + 2 Ae'Ae)
    Kinv via Newton-Schulz (scalar init c*I; K >= 1.1 I by construction)
    M = Kinv At'  (At = [Ai; Ae], 160x128);  G = At M (160x160, symmetric)
    d = At (-Kinv qv)  -> persistent fp32 PSUM bank;  s_vec = -Kinv qv
    Stationary tiles stored bf16: T1 = -G[0:128, 0:160], T2 = -G[128:160, 0:160]
    H = At Kinv (=M^T) spilled to DRAM (fp32) for the final matvec.
  Phase B (98 iterations), state s_t = a_t - u in blocks [i(128); e2(32); e3(32)],
  laid out [m-partitions, element-columns]:
    B   = |rho s|  (fp32 for the exact relu path; bf16 copy feeds the matvec)
    p~  = [B_i ; B_e2-B_e3]
    s' = C' + (0.5/rho) B + 0.5 s - G p~   (+G p~ for the e3 block)
    PSUM accumulates batched identity-stationary matmuls (affine terms, fp32)
    + per-element 4 bf16 matmuls with the stored -G tiles.
  Final: x = M (rho uC - p~_99) + s_vec via the spilled fp32 H as stationary.

Measured on trn2 (8 cores, NTFF profile): 7.27 ms HW exec at the previous
checkpoint, rel err 3.8e-3 vs fp64 reference replica (fp32-everywhere variant:
22.3 ms, rel err 5e-6). Final version additionally moves the per-iteration
affine accumulation off the TensorEngine onto DVE (removes 12 fp32 dual-pass
matmuls per iteration, ~10 us of 97.5 us); HW-verified same rel err.
"""
import os
from collections import deque

import numpy as np

import concourse.bass as bass
import concourse.bacc as bacc
import concourse.mybir as mybir
from concourse.tile import TileContext
from concourse.masks import make_identity
from concourse.bass_utils import run_bass_kernel_spmd

F32 = mybir.dt.float32
BF16 = mybir.dt.bfloat16
ALU = mybir.AluOpType
AFT = mybir.ActivationFunctionType

NCORES = 8
P = 128            # elements per core
NX = 128           # QP dimension
MI = 128           # inequality rows
ME = 32            # equality rows
MT = MI + ME       # 160 collapsed constraint dim

RHO = 0.1
EPS_ = 1e-4
ACOEF = 1.0 + 1e-6          # alpha + sigma added to Q's diagonal
C0 = float(2.0 / (1.1 + 7.5))  # Newton-Schulz scalar init
NS_LOOP = 6                  # NS iterations after the fused first one (7 total)
N_ITER = 100                 # reference ADMM iterations
N_AUPD = N_ITER - 2          # 98 a-state updates (a_1 given, w from a_99)
N_BODY = N_AUPD // 2         # 49 For_i bodies x 2 updates


def _col(t, n):
    return t[:, n:n + 1]


def _strided_cols(t, start, step, count, part=None):
    base = t[:, 0:1] if part is None else t[part[0]:part[1], 0:1]
    return bass.AP(tensor=base.tensor, offset=base.offset + start,
                   ap=[base.ap[0], [step, count]])


def build(n_el=P, n_body=N_BODY, ns_loop=NS_LOOP, taps=False):
    nc = bacc.Bacc()

    x_d = nc.dram_tensor("x", [P, NX, 1], F32, kind="ExternalInput")
    Q_d = nc.dram_tensor("Q", [P, NX, NX], F32, kind="ExternalInput")
    q_d = nc.dram_tensor("q", [P, NX, 1], F32, kind="ExternalInput")
    Ai_d = nc.dram_tensor("A_ineq", [P, MI, NX], F32, kind="ExternalInput")
    bi_d = nc.dram_tensor("b_ineq", [P, MI, 1], F32, kind="ExternalInput")
    Ae_d = nc.dram_tensor("A_eq", [P, ME, NX], F32, kind="ExternalInput")
    be_d = nc.dram_tensor("b_eq", [P, ME, 1], F32, kind="ExternalInput")
    out_d = nc.dram_tensor("out", [P, NX, 1], F32, kind="ExternalOutput")
    hsp_d = nc.dram_tensor("hspill", [P, MT, NX], F32)  # internal DRAM
    if taps:
        dbg_d = nc.dram_tensor("dbg", [8, 128, 256], F32, kind="ExternalOutput")

    with TileContext(nc) as tc:
        with (
            tc.tile_pool(name="consts", bufs=1) as consts,
            tc.tile_pool(name="gpool", bufs=1) as gpool,
            tc.tile_pool(name="work", bufs=4) as work,
            tc.tile_pool(name="wks", bufs=2) as wks,
            tc.tile_pool(name="hre", bufs=8) as hre,
            tc.tile_pool(name="pspool", bufs=8, space="PSUM") as pspool,
        ):
            # ---------------- constants ----------------
            ident = consts.tile([128, 128], F32)
            make_identity(nc, ident)
            negI = consts.tile([128, 128], F32)
            nc.vector.tensor_scalar_mul(negI, ident, -1.0)
            halfI = consts.tile([128, 128], F32)
            nc.vector.tensor_scalar_mul(halfI, ident, 0.5)
            hbrI = consts.tile([128, 128], F32)
            nc.vector.tensor_scalar_mul(hbrI, ident, 0.5 / RHO)
            twoI = consts.tile([128, 128], F32)
            nc.vector.tensor_scalar_mul(twoI, ident, 2.0)
            twoCid = consts.tile([128, 128], F32)
            nc.vector.tensor_scalar_mul(twoCid, ident, 2.0 * C0)
            cIdent = consts.tile([128, 128], F32)
            nc.vector.tensor_scalar_mul(cIdent, ident, ACOEF)
            xinitI = consts.tile([128, 128], F32)
            nc.vector.tensor_scalar_mul(xinitI, ident, 2.0 * C0 - C0 * C0 * ACOEF)
            twoIb = consts.tile([128, 128], BF16)
            nc.vector.tensor_scalar_mul(twoIb, ident, 2.0)

            # ---------------- persistent big tiles ----------------
            Q = n_el // 4  # quads: element n = 4q+k at partition block 32k
            # T1_all: per element -G[0:128, 0:128] bf16, [128, n_el*128]
            T1_all = gpool.tile([128, n_el * 128], BF16)
            # G2A_all: quad-stacked -G[128:160, 0:128] chunks: element 4q+k at
            # partitions 32k, cols q*128..; used as one [128,128] lhsT per quad
            # with a block-sparse rhs.
            G2A_all = gpool.tile([128, Q * 128], BF16)
            # W4_all: quad-packed -G[0:128, 128:160] blocks: element 4q+k in
            # cols q*128+32k.. (M-packed); one [128,128] FWL lhsT per quad,
            # dense rhs cols, element k's result in psum rows 32k.
            W4_all = gpool.tile([128, Q * 128], BF16)
            # G2eD_all: quad block-diagonal -G[128:160, 128:160] blocks:
            # element 4q+k at rows 32k, cols q*128+32k..; zeros elsewhere, so
            # a block-sparse rhs accumulates garbage-free onto W4's rows.
            G2eD_all = gpool.tile([128, Q * 128], BF16)

            def t1(n):
                return T1_all[:, n * 128:(n + 1) * 128]

            def w4(n):
                a, g = n % 4, n // 4
                return W4_all[:, g * 128 + 32 * a:g * 128 + 32 * a + 32]

            def g2ed(n):
                a, g = n % 4, n // 4
                return G2eD_all[32 * a:32 * a + 32,
                                g * 128 + 32 * a:g * 128 + 32 * a + 32]

            # batched constants (m-layout: [m-part, element-cols])
            u_i = gpool.tile([128, n_el], F32)
            be_t = gpool.tile([32, n_el], F32)
            u_e2 = gpool.tile([32, n_el], F32)
            ruC_top = gpool.tile([128, n_el], F32)
            ruC_bot = gpool.tile([32, n_el], F32)
            nruC_top = gpool.tile([128, n_el], BF16)
            nruC_bot = gpool.tile([128, n_el], BF16)   # replicated x4
            nruC_botD = gpool.tile([128, n_el], BF16)  # block-sparse diag scatter
            nqv_all = gpool.tile([128, n_el], F32)
            Cp_i = gpool.tile([128, n_el], F32)
            Cp_e = gpool.tile([32, 2 * n_el], F32)     # [Cp_e2 | Cp_e3]
            S_all = gpool.tile([128, n_el], F32)
            D_all = gpool.tile([128, 2 * n_el], F32)   # [d_top | d_bot(32p)]
            # ADMM state (ping-pong a/b)
            s_i = [gpool.tile([128, n_el], F32, name=f"s_i{j}") for j in range(2)]
            s_e = [gpool.tile([32, 2 * n_el], F32, name=f"s_e{j}") for j in range(2)]
            B_i = [gpool.tile([128, n_el], F32, name=f"B_i{j}") for j in range(2)]
            B_e = [gpool.tile([32, 2 * n_el], F32, name=f"B_e{j}") for j in range(2)]
            Bib = [gpool.tile([128, n_el], BF16, name=f"Bib{j}") for j in range(2)]
            pbot = [gpool.tile([128, n_el], BF16, name=f"pbot{j}") for j in range(2)]
            pbotD = [gpool.tile([128, n_el], BF16, name=f"pbotD{j}") for j in range(2)]
            he_sb = [gpool.tile([32, n_el], F32, name=f"he_sb{j}") for j in range(2)]
            f_top = gpool.tile([128, n_el], F32)
            f_bot = gpool.tile([32, n_el], F32)
            xo = gpool.tile([128, n_el], F32)
            xout = gpool.tile([n_el, 128], F32)

            nc.vector.memset(pbotD[0], 0.0)
            nc.vector.memset(pbotD[1], 0.0)
            nc.vector.memset(nruC_botD, 0.0)
            nc.vector.memset(G2eD_all, 0.0)

            # ---------------- batched input prep ----------------
            x_el = wks.tile([P, NX], F32, tag="xel")
            q_el = wks.tile([P, NX], F32, tag="qel")
            nc.sync.dma_start(out=x_el, in_=x_d[:, :, 0])
            nc.sync.dma_start(out=q_el, in_=q_d[:, :, 0])
            nq_el = wks.tile([P, NX], F32, tag="nqel")
            nc.vector.tensor_tensor(nq_el, x_el, q_el, ALU.subtract)  # -(q - x)
            nqps = pspool.tile([128, P], F32, tag="ps")
            nc.tensor.transpose(nqps, nq_el, ident)
            nc.vector.tensor_copy(nqv_all, nqps[:, 0:n_el])

            bi_el = wks.tile([P, MI], F32, tag="biel")
            nc.sync.dma_start(out=bi_el, in_=bi_d[:, :, 0])
            bips = pspool.tile([128, P], F32, tag="ps")
            nc.tensor.transpose(bips, bi_el, ident)
            nc.vector.tensor_copy(u_i, bips[:, 0:n_el])

            be_el = wks.tile([P, ME], F32, tag="beel")
            nc.sync.dma_start(out=be_el, in_=be_d[:, :, 0])
            beps = pspool.tile([32, P], F32, tag="ps")
            nc.tensor.transpose(beps, be_el, ident)
            nc.vector.tensor_copy(be_t, beps[:, 0:n_el])

            nc.vector.tensor_scalar_add(u_e2, be_t, EPS_)
            nc.vector.tensor_scalar_mul(ruC_top, u_i, RHO)
            nc.vector.tensor_scalar(out=ruC_bot, in0=be_t, scalar1=2.0 * RHO,
                                    scalar2=RHO * EPS_, op0=ALU.mult, op1=ALU.add)
            nc.vector.tensor_scalar_mul(nruC_top, u_i, -RHO)
            nc.vector.tensor_scalar(out=nruC_bot[0:32, :], in0=be_t,
                                    scalar1=-2.0 * RHO, scalar2=-RHO * EPS_,
                                    op0=ALU.mult, op1=ALU.add)
            nc.vector.tensor_copy(nruC_bot[32:64, :], nruC_bot[0:32, :])
            nc.vector.tensor_copy(nruC_bot[64:128, :], nruC_bot[0:64, :])
            for k in range(4):
                nc.vector.tensor_copy(
                    _strided_cols(nruC_botD, k, 4, Q, part=(32 * k, 32 * k + 32)),
                    _strided_cols(nruC_bot, k, 4, Q, part=(32 * k, 32 * k + 32)))

            # ---------------- phase A: per-element factorization ----------------
            # Emitted as a K_PIPE-way software pipeline: each element's chain
            # is a staged generator and stages of neighbouring elements are
            # interleaved in issue order, so the strict-FIFO ACT/DVE queues
            # overlap work across elements instead of head-of-line blocking.
            SQ2 = float(np.sqrt(2.0))

            def elem_stages(n):
                a_, q_ = n % 4, n // 4
                Qt = work.tile([128, 128], F32, tag="Q")
                nc.sync.dma_start(out=Qt, in_=Q_d[n])
                Ait = work.tile([128, 128], F32, tag="Ai")
                nc.sync.dma_start(out=Ait, in_=Ai_d[n])
                Aet = work.tile([32, 128], F32, tag="Ae")
                nc.sync.dma_start(out=Aet, in_=Ae_d[n])
                yield

                at_ps = pspool.tile([128, MT], F32, tag="ps")
                nc.tensor.transpose(at_ps[:, 0:128], Ait, ident)
                nc.tensor.transpose(at_ps[:, 128:160], Aet, ident[0:32, 0:32])
                AT = work.tile([128, MT], F32, tag="AT")
                nc.vector.tensor_copy(AT, at_ps)
                Aib = work.tile([128, 128], BF16, tag="Aib")
                nc.vector.tensor_copy(Aib, Ait)
                Ae2 = work.tile([32, 128], BF16, tag="Ae2")
                nc.vector.tensor_scalar_mul(Ae2, Aet, SQ2)
                yield

                # K = Q + (alpha+sigma) I + rho (Ai'Ai + 2 Ae'Ae); the rho
                # factor is folded into the psum consumer so only unscaled
                # bf16 casts of Ai / sqrt(2) Ae are needed.
                K_ps = pspool.tile([128, 128], F32, tag="ps")
                nc.tensor.matmul(K_ps, Aib, Aib, start=True, stop=False)
                nc.tensor.matmul(K_ps, Ae2, Ae2, start=False, stop=True)
                tmp = work.tile([128, 128], F32, tag="tmp")
                nc.vector.scalar_tensor_tensor(out=tmp, in0=K_ps, scalar=-RHO,
                                               in1=Qt, op0=ALU.mult,
                                               op1=ALU.subtract)
                negK = work.tile([128, 128], F32, tag="negK")
                nc.vector.scalar_tensor_tensor(out=negK, in0=tmp, scalar=1.0,
                                               in1=cIdent, op0=ALU.mult,
                                               op1=ALU.subtract)
                negKb = work.tile([128, 128], BF16, tag="negKb")
                nc.vector.tensor_copy(negKb, negK)
                # NS state lives in bf16 (operand precision); the final fp32
                # polish squares away the bf16 state floor.
                Xb = work.tile([128, 128], BF16, tag="Xb", bufs=8)
                nc.vector.scalar_tensor_tensor(out=Xb, in0=tmp, scalar=C0 * C0,
                                               in1=xinitI, op0=ALU.mult,
                                               op1=ALU.add)
                yield

                for k in range(ns_loop - 2):
                    G1_ps = pspool.tile([128, 128], F32, tag="ps")
                    nc.tensor.matmul(G1_ps, negKb, Xb, start=True, stop=True)
                    g1 = work.tile([128, 128], BF16, tag="g1", bufs=8)
                    nc.scalar.activation(g1, G1_ps, AFT.Copy)
                    X2_ps = pspool.tile([128, 128], F32, tag="ps")
                    nc.tensor.matmul(X2_ps, Xb, g1, start=True, stop=True)
                    Xn = work.tile([128, 128], BF16, tag="Xb", bufs=8)
                    nc.vector.scalar_tensor_tensor(out=Xn, in0=Xb, scalar=2.0,
                                                   in1=X2_ps, op0=ALU.mult,
                                                   op1=ALU.add)
                    Xb = Xn
                    yield

                # fp32 polish iteration: X8 = 2 Xf + Xf negK Xf with Xf the
                # upcast bf16 state; explicit transpose feeds lhsT since Xf is
                # not exactly symmetric.
                XfF = work.tile([128, 128], F32, tag="XfF")
                nc.vector.tensor_copy(XfF, Xb)
                XfT_ps = pspool.tile([128, 128], F32, tag="ps")
                nc.tensor.transpose(XfT_ps, XfF, ident)
                XfT = work.tile([128, 128], F32, tag="XfT")
                nc.vector.tensor_copy(XfT, XfT_ps)
                G1p = pspool.tile([128, 128], F32, tag="ps")
                nc.tensor.matmul(G1p, negK, XfF, start=True, stop=True)
                g1f = work.tile([128, 128], F32, tag="g1f")
                nc.scalar.activation(g1f, G1p, AFT.Copy)
                X2p = pspool.tile([128, 128], F32, tag="ps")
                nc.tensor.matmul(X2p, XfT, g1f, start=True, stop=False)
                nc.tensor.matmul(X2p, twoI, XfF, start=False, stop=True)
                X = work.tile([128, 128], F32, tag="X8")
                nc.vector.tensor_copy(X, X2p)
                yield

                # M = Kinv At'
                Ms_ps = pspool.tile([128, MT], F32, tag="ps")
                nc.tensor.matmul(Ms_ps, X, AT, start=True, stop=True)
                Ms = work.tile([128, MT], F32, tag="Ms")
                nc.vector.tensor_copy(Ms, Ms_ps)
                ATb = work.tile([128, MT], BF16, tag="ATb")
                nc.vector.tensor_copy(ATb, AT)
                Msb = work.tile([128, MT], BF16, tag="Msb")
                nc.vector.tensor_copy(Msb, Ms)
                yield

                # H = At Kinv = Ms^T via PE transposes (single-pass); the
                # s_vec / d columns ride in spare columns of the same bank so
                # no dedicated PSUM banks are pinned across phase A.
                H_ps = pspool.tile([128, 260], F32, tag="ps")
                nc.tensor.transpose(H_ps[:, 0:128], Ms[:, 0:128], ident)
                nc.tensor.transpose(H_ps[0:32, 128:256], Ms[:, 128:160],
                                    ident)
                nc.tensor.matmul(H_ps[:, 256:257], X, _col(nqv_all, n),
                                 start=True, stop=True, skip_group_check=True)
                nc.tensor.matmul(H_ps[:, 257:258], Ms[:, 0:128],
                                 _col(nqv_all, n),
                                 start=True, stop=True, skip_group_check=True)
                nc.tensor.matmul(H_ps[0:32, 258:259], Ms[:, 128:160],
                                 _col(nqv_all, n),
                                 start=True, stop=True, skip_group_check=True)
                Gr1_ps = pspool.tile([128, MT], F32, tag="ps")
                nc.tensor.matmul(Gr1_ps, ATb[:, 0:128], Msb, start=True,
                                 stop=True)
                Gr2_ps = pspool.tile([32, MT], F32, tag="ps")
                nc.tensor.matmul(Gr2_ps, ATb[:, 128:160], Msb, start=True,
                                 stop=True)
                yield

                nc.vector.tensor_copy(_col(S_all, n), H_ps[:, 256:257])
                nc.vector.tensor_copy(_col(D_all, n), H_ps[:, 257:258])
                nc.vector.tensor_copy(D_all[0:32, n_el + n:n_el + n + 1],
                                      H_ps[0:32, 258:259])
                nc.vector.tensor_scalar_mul(t1(n), Gr1_ps[:, 0:128], -1.0)
                nc.vector.tensor_scalar_mul(w4(n), Gr1_ps[:, 128:160], -1.0)
                nc.vector.tensor_scalar_mul(
                    G2A_all[32 * a_:32 * a_ + 32, q_ * 128:(q_ + 1) * 128],
                    Gr2_ps[:, 0:128], -1.0)
                nc.vector.tensor_scalar_mul(g2ed(n), Gr2_ps[:, 128:160], -1.0)
                Htile = work.tile([128, 256], F32, tag="H")
                nc.scalar.activation(Htile[:, 0:128], H_ps[:, 0:128], AFT.Copy)
                nc.scalar.activation(Htile[0:32, 128:256], H_ps[0:32, 128:256],
                                     AFT.Copy)
                nc.sync.dma_start(out=hsp_d[n, 0:128, :], in_=Htile[:, 0:128])
                nc.sync.dma_start(out=hsp_d[n, 128:160, :],
                                  in_=Htile[0:32, 128:256])

                if taps and n == 0:
                    nc.sync.dma_start(out=dbg_d[0, :, 0:128], in_=negK)
                    nc.sync.dma_start(out=dbg_d[1, :, 0:128], in_=X)
                    nc.sync.dma_start(out=dbg_d[2, :, 0:MT], in_=Ms)
                    nc.sync.dma_start(out=dbg_d[3, :, 0:128], in_=XfF)
                    nc.sync.dma_start(out=dbg_d[4, :, 0:128], in_=g1f)

            K_PIPE = 3
            pend = deque()
            nextn = 0
            while pend or nextn < n_el:
                while len(pend) < K_PIPE and nextn < n_el:
                    pend.append(elem_stages(nextn))
                    nextn += 1
                g = pend.popleft()
                try:
                    next(g)
                    pend.append(g)
                except StopIteration:
                    pass

            # ---------------- s1 init + C' prepass ----------------
            # top: psum = d_i - u_i (s1), then + g0_i (C')
            S1T = pspool.tile([128, n_el], F32, tag="ps")
            nc.tensor.matmul(S1T, negI, u_i, start=True, stop=False,
                             skip_group_check=True)
            nc.tensor.matmul(S1T, ident, D_all[:, 0:n_el], start=False, stop=False,
                             skip_group_check=True)
            nc.vector.tensor_copy(s_i[0], S1T)
            S1E = pspool.tile([32, n_el], F32, tag="ps")
            nc.tensor.matmul(S1E, negI[0:32, 0:32], u_e2, start=True, stop=False,
                             skip_group_check=True)
            nc.tensor.matmul(S1E, ident[0:32, 0:32],
                             D_all[0:32, n_el:2 * n_el], start=False, stop=True,
                             skip_group_check=True)
            nc.vector.tensor_copy(s_e[0][:, 0:n_el], S1E)
            nc.vector.tensor_scalar(out=s_e[0][:, n_el:2 * n_el], in0=S1E,
                                    scalar1=-1.0, scalar2=-EPS_,
                                    op0=ALU.mult, op1=ALU.add)

            # accumulate g0 terms (bf16 G x bf16 -rho*uC) into the psums;
            # the e-block terms go through the quad-packed scratch.
            scr0 = pspool.tile([128, n_el], F32, tag="ps")
            for n in range(n_el):
                nc.tensor.matmul(_col(S1T, n), t1(n), _col(nruC_top, n),
                                 start=False, stop=False, skip_group_check=True)
            for q in range(Q):
                nc.tensor.matmul(S1T[:, 4 * q:4 * q + 4],
                                 G2A_all[:, q * 128:(q + 1) * 128],
                                 nruC_botD[:, 4 * q:4 * q + 4],
                                 start=False, stop=(q == Q - 1),
                                 skip_group_check=True)
            for q in range(Q):
                nc.tensor.matmul(scr0[:, 4 * q:4 * q + 4],
                                 W4_all[:, q * 128:(q + 1) * 128],
                                 nruC_top[:, 4 * q:4 * q + 4],
                                 start=(q == 0), stop=False,
                                 skip_group_check=True)
            for q in range(Q):
                nc.tensor.matmul(scr0[:, 4 * q:4 * q + 4],
                                 G2eD_all[:, q * 128:(q + 1) * 128],
                                 nruC_botD[:, 4 * q:4 * q + 4],
                                 start=False, stop=(q == Q - 1),
                                 skip_group_check=True)
            nc.vector.tensor_copy(Cp_i, S1T)
            for a in range(4):
                nc.vector.tensor_copy(
                    _strided_cols(he_sb[0], a, 4, Q),
                    _strided_cols(scr0, a, 4, Q, part=(32 * a, 32 * a + 32)))
            nc.vector.tensor_tensor(Cp_e[:, 0:n_el], s_e[0][:, 0:n_el],
                                    he_sb[0], ALU.add)
            nc.vector.tensor_scalar(out=Cp_e[:, n_el:2 * n_el],
                                    in0=Cp_e[:, 0:n_el],
                                    scalar1=-1.0, scalar2=-EPS_,
                                    op0=ALU.mult, op1=ALU.add)
            if taps:
                nc.sync.dma_start(out=dbg_d[5, :, 0:n_el], in_=Cp_i)
                nc.sync.dma_start(out=dbg_d[6, :, 0:n_el], in_=s_i[0])

            # ---------------- phase B: ADMM loop ----------------
            # Each half_iter is split into two independent 64-element halves;
            # half h's DVE state-update chain overlaps the other half's PE
            # block, so steady state is pure PE (weight-load bound).  B/Bib/
            # pbot/pbotD for a state index are computed as soon as that state
            # is produced (epilogue of the producing half), so the next PE
            # block never waits on DVE.
            HQ = Q // 2  # quads per half

            def state_tail(dst, h):
                """B / Bib for the i-block of half h of state dst."""
                hs = slice(64 * h, 64 * h + 64)
                nc.scalar.activation(B_i[dst][:, hs], s_i[dst][:, hs],
                                     AFT.Abs, scale=RHO)
                nc.scalar.activation(Bib[dst][:, hs], B_i[dst][:, hs],
                                     AFT.Copy)

            def state_tail_e(dst):
                """B_e / pbot / pbotD for state dst (full width)."""
                nc.scalar.activation(B_e[dst], s_e[dst], AFT.Abs, scale=RHO)
                nc.vector.tensor_tensor(pbot[dst][0:32, :],
                                        B_e[dst][:, 0:n_el],
                                        B_e[dst][:, n_el:2 * n_el],
                                        ALU.subtract)
                nc.vector.tensor_copy(pbot[dst][32:64, :], pbot[dst][0:32, :])
                nc.vector.tensor_copy(pbot[dst][64:128, :], pbot[dst][0:64, :])
                for k in range(4):
                    nc.vector.tensor_copy(
                        _strided_cols(pbotD[dst], k, 4, Q,
                                      part=(32 * k, 32 * k + 32)),
                        _strided_cols(pbot[dst], k, 4, Q,
                                      part=(32 * k, 32 * k + 32)))

            def half_iter(src, dst):
                # e-chain prefix only reads src state: issue up front.
                u1 = wks.tile([32, 2 * n_el], F32, tag="u1")
                nc.vector.scalar_tensor_tensor(out=u1, in0=B_e[src],
                                               scalar=0.5 / RHO, in1=Cp_e,
                                               op0=ALU.mult, op1=ALU.add)
                u2 = wks.tile([32, 2 * n_el], F32, tag="u2")
                nc.vector.scalar_tensor_tensor(out=u2, in0=s_e[src],
                                               scalar=0.5, in1=u1,
                                               op0=ALU.mult, op1=ALU.add)
                banks = []
                for h in (0, 1):
                    hs = slice(64 * h, 64 * h + 64)
                    bankT = pspool.tile([128, 64], F32, tag="ps")
                    scr = pspool.tile([128, 64], F32, tag="ps")
                    banks.append((bankT, scr))
                    for j in range(64):
                        n = 64 * h + j
                        nc.tensor.matmul(_col(bankT, j), t1(n),
                                         _col(Bib[src], n), start=(j == 0),
                                         stop=False, skip_group_check=True)
                    for i in range(HQ):
                        q = HQ * h + i
                        nc.tensor.matmul(bankT[:, 4 * i:4 * i + 4],
                                         G2A_all[:, q * 128:(q + 1) * 128],
                                         pbotD[src][:, 4 * q:4 * q + 4],
                                         start=False, stop=(i == HQ - 1),
                                         skip_group_check=True)
                    for i in range(HQ):
                        q = HQ * h + i
                        nc.tensor.matmul(scr[:, 4 * i:4 * i + 4],
                                         W4_all[:, q * 128:(q + 1) * 128],
                                         Bib[src][:, 4 * q:4 * q + 4],
                                         start=(i == 0), stop=False,
                                         skip_group_check=True)
                    for i in range(HQ):
                        q = HQ * h + i
                        nc.tensor.matmul(scr[:, 4 * i:4 * i + 4],
                                         G2eD_all[:, q * 128:(q + 1) * 128],
                                         pbotD[src][:, 4 * q:4 * q + 4],
                                         start=False, stop=(i == HQ - 1),
                                         skip_group_check=True)
                    # s_i' chain for half h (overlaps the other half's PE)
                    t1x = wks.tile([128, 64], F32, tag=f"t1x{h}")
                    nc.vector.scalar_tensor_tensor(out=t1x,
                                                   in0=B_i[src][:, hs],
                                                   scalar=0.5 / RHO,
                                                   in1=Cp_i[:, hs],
                                                   op0=ALU.mult, op1=ALU.add)
                    t2x = wks.tile([128, 64], F32, tag=f"t2x{h}")
                    nc.vector.scalar_tensor_tensor(out=t2x,
                                                   in0=s_i[src][:, hs],
                                                   scalar=0.5, in1=bankT,
                                                   op0=ALU.mult, op1=ALU.add)
                    nc.vector.tensor_tensor(s_i[dst][:, hs], t1x, t2x,
                                            ALU.add)
                    state_tail(dst, h)
                    # he gather for half h from its scratch psum
                    for a in range(4):
                        nc.vector.tensor_copy(
                            _strided_cols(he_sb[src], 64 * h + a, 4, HQ),
                            _strided_cols(scr, a, 4, HQ,
                                          part=(32 * a, 32 * a + 32)))
                # e-state update (full width) + its tail
                nc.vector.tensor_tensor(s_e[dst][:, 0:n_el],
                                        u2[:, 0:n_el], he_sb[src], ALU.add)
                nc.vector.tensor_tensor(s_e[dst][:, n_el:2 * n_el],
                                        u2[:, n_el:2 * n_el],
                                        he_sb[src], ALU.subtract)
                state_tail_e(dst)

            # prologue: derived state for s1
            state_tail(0, 0)
            state_tail(0, 1)
            state_tail_e(0)
            if n_body > 0:
                with tc.For_i(0, n_body, 1,
                              hint_engines=(mybir.EngineType.PE,)):
                    half_iter(0, 1)
                    half_iter(1, 0)

            # ---------------- final: x = M (rho uC - p~_99) + s_vec -------------
            # B_i[0] / B_e[0] are already maintained by the state tails.
            nc.vector.tensor_tensor(f_bot, B_e[0][:, 0:n_el],
                                    B_e[0][:, n_el:2 * n_el], ALU.subtract)
            nc.vector.tensor_tensor(f_bot, ruC_bot, f_bot, ALU.subtract)
            nc.vector.tensor_tensor(f_top, ruC_top, B_i[0], ALU.subtract)

            xP = pspool.tile([128, n_el], F32, tag="ps")
            nc.tensor.matmul(xP, ident, S_all, start=True, stop=False,
                             skip_group_check=True)
            for n in range(n_el):
                Ht = hre.tile([128, 128], F32, tag="hret")
                nc.sync.dma_start(out=Ht, in_=hsp_d[n, 0:128, :])
                Hb = hre.tile([32, 128], F32, tag="hreb")
                nc.sync.dma_start(out=Hb, in_=hsp_d[n, 128:160, :])
                nc.tensor.matmul(_col(xP, n), Ht, _col(f_top, n),
                                 start=False, stop=False, skip_group_check=True)
                nc.tensor.matmul(_col(xP, n), Hb, _col(f_bot, n),
                                 start=False, stop=(n == n_el - 1),
                                 skip_group_check=True)
            nc.vector.tensor_copy(xo, xP)
            if taps:
                nc.sync.dma_start(out=dbg_d[7, :, 0:n_el], in_=s_i[0])
            xT = pspool.tile([n_el, 128], F32, tag="ps")
            nc.tensor.transpose(xT, xo, ident)
            nc.vector.tensor_copy(xout, xT)
            nc.sync.dma_start(out=out_d[0:n_el, :, 0], in_=xout)

    nc.compile()
    return nc


_NC_CACHE = {}


def _get_nc(taps=False):
    key = taps
    if key not in _NC_CACHE:
        _NC_CACHE[key] = build(taps=taps)
    return _NC_CACHE[key]


def run(inputs, taps=False, trace=False):
    nc = _get_nc(taps=taps)
    in_maps = []
    for c in range(NCORES):
        sl = slice(c * P, (c + 1) * P)
        in_maps.append({k: np.ascontiguousarray(np.asarray(v)[sl], dtype=np.float32)
                        for k, v in inputs.items()})
    res = run_bass_kernel_spmd(nc, in_maps, core_ids=list(range(NCORES)),
                               trace=trace)
    out = np.concatenate([res.results[c]["out"] for c in range(NCORES)], axis=0)
    return out, res


def kernel(**inputs):
    out, _ = run(inputs)
    return out



# revision 20
# speedup vs baseline: 1.0574x; 1.0574x over previous
"""Trainium2 Bass kernel for nn_BackwardStep_38749194944853.

Batched ADMM QP solve (OSQP-style), N=1024 independent QPs of dim nx=128 with
mi=128 inequality + me=32 doubled equality constraints, 100 fixed iterations.

Strategy (pure data-parallel over 8 cores, 128 QPs per core):
  Phase A (per element, TensorE-heavy):
    K = Q + (1+sigma) I + rho (Ai'Ai + 2 Ae'Ae)
    Kinv via Newton-Schulz (scalar init c*I; K >= 1.1 I by construction)
    M = Kinv At'  (At = [Ai; Ae], 160x128);  G = At M (160x160, symmetric)
    d = At (-Kinv qv)  -> persistent fp32 PSUM bank;  s_vec = -Kinv qv
    Stationary tiles stored bf16: T1 = -G[0:128, 0:160], T2 = -G[128:160, 0:160]
    H = At Kinv (=M^T) spilled to DRAM (fp32) for the final matvec.
  Phase B (98 iterations), state s_t = a_t - u in blocks [i(128); e2(32); e3(32)],
  laid out [m-partitions, element-columns]:
    B   = |rho s|  (fp32 for the exact relu path; bf16 copy feeds the matvec)
    p~  = [B_i ; B_e2-B_e3]
    s' = C' + (0.5/rho) B + 0.5 s - G p~   (+G p~ for the e3 block)
    PSUM accumulates batched identity-stationary matmuls (affine terms, fp32)
    + per-element 4 bf16 matmuls with the stored -G tiles.
  Final: x = M (rho uC - p~_99) + s_vec via the spilled fp32 H as stationary.

Measured on trn2 (8 cores, NTFF profile): 7.27 ms HW exec at the previous
checkpoint, rel err 3.8e-3 vs fp64 reference replica (fp32-everywhere variant:
22.3 ms, rel err 5e-6). Final version additionally moves the per-iteration
affine accumulation off the TensorEngine onto DVE (removes 12 fp32 dual-pass
matmuls per iteration, ~10 us of 97.5 us); HW-verified same rel err.
"""
import os
from collections import deque

import numpy as np

import concourse.bass as bass
import concourse.bacc as bacc
import concourse.mybir as mybir
from concourse.tile import TileContext
from concourse.masks import make_identity
from concourse.bass_utils import run_bass_kernel_spmd

F32 = mybir.dt.float32
BF16 = mybir.dt.bfloat16
ALU = mybir.AluOpType
AFT = mybir.ActivationFunctionType

NCORES = 8
P = 128            # elements per core
NX = 128           # QP dimension
MI = 128           # inequality rows
ME = 32            # equality rows
MT = MI + ME       # 160 collapsed constraint dim

RHO = 0.1
EPS_ = 1e-4
ACOEF = 1.0 + 1e-6          # alpha + sigma added to Q's diagonal
C0 = float(2.0 / (1.1 + 7.5))  # Newton-Schulz scalar init
NS_LOOP = 5                  # NS iterations after the fused first one (6 total)
N_ITER = 100                 # reference ADMM iterations
N_AUPD = N_ITER - 2          # 98 a-state updates (a_1 given, w from a_99)
N_BODY = N_AUPD // 2         # 49 For_i bodies x 2 updates


def _col(t, n):
    return t[:, n:n + 1]


def _strided_cols(t, start, step, count, part=None):
    base = t[:, 0:1] if part is None else t[part[0]:part[1], 0:1]
    return bass.AP(tensor=base.tensor, offset=base.offset + start,
                   ap=[base.ap[0], [step, count]])


def build(n_el=P, n_body=N_BODY, ns_loop=NS_LOOP, taps=False):
    nc = bacc.Bacc()

    x_d = nc.dram_tensor("x", [P, NX, 1], F32, kind="ExternalInput")
    Q_d = nc.dram_tensor("Q", [P, NX, NX], F32, kind="ExternalInput")
    q_d = nc.dram_tensor("q", [P, NX, 1], F32, kind="ExternalInput")
    Ai_d = nc.dram_tensor("A_ineq", [P, MI, NX], F32, kind="ExternalInput")
    bi_d = nc.dram_tensor("b_ineq", [P, MI, 1], F32, kind="ExternalInput")
    Ae_d = nc.dram_tensor("A_eq", [P, ME, NX], F32, kind="ExternalInput")
    be_d = nc.dram_tensor("b_eq", [P, ME, 1], F32, kind="ExternalInput")
    out_d = nc.dram_tensor("out", [P, NX, 1], F32, kind="ExternalOutput")
    hsp_d = nc.dram_tensor("hspill", [P, MT, NX], BF16)  # internal DRAM
    if taps:
        dbg_d = nc.dram_tensor("dbg", [8, 128, 256], F32, kind="ExternalOutput")

    with TileContext(nc) as tc:
        with (
            tc.tile_pool(name="consts", bufs=1) as consts,
            tc.tile_pool(name="gpool", bufs=1) as gpool,
            tc.tile_pool(name="work", bufs=4) as work,
            tc.tile_pool(name="wks", bufs=2) as wks,
            tc.tile_pool(name="hre", bufs=8) as hre,
            tc.tile_pool(name="pspool", bufs=8, space="PSUM") as pspool,
        ):
            # ---------------- constants ----------------
            ident = consts.tile([128, 128], F32)
            make_identity(nc, ident)
            negI = consts.tile([128, 128], F32)
            nc.vector.tensor_scalar_mul(negI, ident, -1.0)
            halfI = consts.tile([128, 128], F32)
            nc.vector.tensor_scalar_mul(halfI, ident, 0.5)
            hbrI = consts.tile([128, 128], F32)
            nc.vector.tensor_scalar_mul(hbrI, ident, 0.5 / RHO)
            twoI = consts.tile([128, 128], F32)
            nc.vector.tensor_scalar_mul(twoI, ident, 2.0)
            twoCid = consts.tile([128, 128], F32)
            nc.vector.tensor_scalar_mul(twoCid, ident, 2.0 * C0)
            cIdent = consts.tile([128, 128], F32)
            nc.vector.tensor_scalar_mul(cIdent, ident, ACOEF)
            xinitI = consts.tile([128, 128], F32)
            nc.vector.tensor_scalar_mul(xinitI, ident, 2.0 * C0 - C0 * C0 * ACOEF)
            twoIb = consts.tile([128, 128], BF16)
            nc.vector.tensor_scalar_mul(twoIb, ident, 2.0)
            identb = consts.tile([128, 128], BF16)
            nc.vector.tensor_copy(identb, ident)

            # ---------------- persistent big tiles ----------------
            Q = n_el // 4  # quads: element n = 4q+k at partition block 32k
            # T1_all: per element -G[0:128, 0:128] bf16, [128, n_el*128]
            T1_all = gpool.tile([128, n_el * 128], BF16)
            # G2A_all: quad-stacked -G[128:160, 0:128] chunks: element 4q+k at
            # partitions 32k, cols q*128..; used as one [128,128] lhsT per quad
            # with a block-sparse rhs.
            G2A_all = gpool.tile([128, Q * 128], BF16)
            # W4_all: quad-packed -G[0:128, 128:160] blocks: element 4q+k in
            # cols q*128+32k.. (M-packed); one [128,128] FWL lhsT per quad,
            # dense rhs cols, element k's result in psum rows 32k.
            W4_all = gpool.tile([128, Q * 128], BF16)
            # G2eD_all: quad block-diagonal -G[128:160, 128:160] blocks:
            # element 4q+k at rows 32k, cols q*128+32k..; zeros elsewhere, so
            # a block-sparse rhs accumulates garbage-free onto W4's rows.
            G2eD_all = gpool.tile([128, Q * 128], BF16)

            def t1(n):
                return T1_all[:, n * 128:(n + 1) * 128]

            def w4(n):
                a, g = n % 4, n // 4
                return W4_all[:, g * 128 + 32 * a:g * 128 + 32 * a + 32]

            def g2ed(n):
                a, g = n % 4, n // 4
                return G2eD_all[32 * a:32 * a + 32,
                                g * 128 + 32 * a:g * 128 + 32 * a + 32]

            # batched constants (m-layout: [m-part, element-cols])
            u_i = gpool.tile([128, n_el], F32)
            be_t = gpool.tile([32, n_el], F32)
            u_e2 = gpool.tile([32, n_el], F32)
            ruC_top = gpool.tile([128, n_el], F32)
            ruC_bot = gpool.tile([32, n_el], F32)
            nruC_top = gpool.tile([128, n_el], BF16)
            nruC_bot = gpool.tile([128, n_el], BF16)   # replicated x4
            nruC_botD = gpool.tile([128, n_el], BF16)  # block-sparse diag scatter
            nqv_all = gpool.tile([128, n_el], F32)
            Cp_i = gpool.tile([128, n_el], F32)
            Cp_e = gpool.tile([32, 2 * n_el], F32)     # [Cp_e2 | Cp_e3]
            S_all = gpool.tile([128, n_el], F32)
            D_all = gpool.tile([128, 2 * n_el], F32)   # [d_top | d_bot(32p)]
            # ADMM state (ping-pong a/b)
            s_i = [gpool.tile([128, n_el], F32, name=f"s_i{j}") for j in range(2)]
            s_e = [gpool.tile([32, 2 * n_el], F32, name=f"s_e{j}") for j in range(2)]
            B_i = [gpool.tile([128, n_el], F32, name=f"B_i{j}") for j in range(2)]
            B_e = [gpool.tile([32, 2 * n_el], F32, name=f"B_e{j}") for j in range(2)]
            Bib = [gpool.tile([128, n_el], BF16, name=f"Bib{j}") for j in range(2)]
            pbot = [gpool.tile([128, n_el], BF16, name=f"pbot{j}") for j in range(2)]
            pbotD = [gpool.tile([128, n_el], BF16, name=f"pbotD{j}") for j in range(2)]
            he_sb = [gpool.tile([32, n_el], F32, name=f"he_sb{j}") for j in range(2)]
            f_top = gpool.tile([128, n_el], F32)
            f_bot = gpool.tile([32, n_el], F32)
            xo = gpool.tile([128, n_el], F32)
            xout = gpool.tile([n_el, 128], F32)

            nc.vector.memset(pbotD[0], 0.0)
            nc.vector.memset(pbotD[1], 0.0)
            nc.vector.memset(nruC_botD, 0.0)
            nc.vector.memset(G2eD_all, 0.0)

            # ---------------- batched input prep ----------------
            x_el = wks.tile([P, NX], F32, tag="xel")
            q_el = wks.tile([P, NX], F32, tag="qel")
            nc.sync.dma_start(out=x_el, in_=x_d[:, :, 0])
            nc.sync.dma_start(out=q_el, in_=q_d[:, :, 0])
            nq_el = wks.tile([P, NX], F32, tag="nqel")
            nc.vector.tensor_tensor(nq_el, x_el, q_el, ALU.subtract)  # -(q - x)
            nqps = pspool.tile([128, P], F32, tag="ps")
            nc.tensor.transpose(nqps, nq_el, ident)
            nc.vector.tensor_copy(nqv_all, nqps[:, 0:n_el])

            bi_el = wks.tile([P, MI], F32, tag="biel")
            nc.sync.dma_start(out=bi_el, in_=bi_d[:, :, 0])
            bips = pspool.tile([128, P], F32, tag="ps")
            nc.tensor.transpose(bips, bi_el, ident)
            nc.vector.tensor_copy(u_i, bips[:, 0:n_el])

            be_el = wks.tile([P, ME], F32, tag="beel")
            nc.sync.dma_start(out=be_el, in_=be_d[:, :, 0])
            beps = pspool.tile([32, P], F32, tag="ps")
            nc.tensor.transpose(beps, be_el, ident)
            nc.vector.tensor_copy(be_t, beps[:, 0:n_el])

            nc.vector.tensor_scalar_add(u_e2, be_t, EPS_)
            nc.vector.tensor_scalar_mul(ruC_top, u_i, RHO)
            nc.vector.tensor_scalar(out=ruC_bot, in0=be_t, scalar1=2.0 * RHO,
                                    scalar2=RHO * EPS_, op0=ALU.mult, op1=ALU.add)
            nc.vector.tensor_scalar_mul(nruC_top, u_i, -RHO)
            nc.vector.tensor_scalar(out=nruC_bot[0:32, :], in0=be_t,
                                    scalar1=-2.0 * RHO, scalar2=-RHO * EPS_,
                                    op0=ALU.mult, op1=ALU.add)
            nc.vector.tensor_copy(nruC_bot[32:64, :], nruC_bot[0:32, :])
            nc.vector.tensor_copy(nruC_bot[64:128, :], nruC_bot[0:64, :])
            for k in range(4):
                nc.vector.tensor_copy(
                    _strided_cols(nruC_botD, k, 4, Q, part=(32 * k, 32 * k + 32)),
                    _strided_cols(nruC_bot, k, 4, Q, part=(32 * k, 32 * k + 32)))

            # ---------------- phase A: per-element factorization ----------------
            # Emitted as a K_PIPE-way software pipeline: each element's chain
            # is a staged generator and stages of neighbouring elements are
            # interleaved in issue order, so the strict-FIFO ACT/DVE queues
            # overlap work across elements instead of head-of-line blocking.
            SQ2 = float(np.sqrt(2.0))

            def elem_stages(n):
                a_, q_ = n % 4, n // 4
                Qt = work.tile([128, 128], F32, tag="Q")
                nc.sync.dma_start(out=Qt, in_=Q_d[n])
                Ait = work.tile([128, 128], F32, tag="Ai")
                nc.sync.dma_start(out=Ait, in_=Ai_d[n])
                Aet = work.tile([32, 128], F32, tag="Ae")
                nc.sync.dma_start(out=Aet, in_=Ae_d[n])
                yield

                Aib = work.tile([128, 128], BF16, tag="Aib")
                nc.vector.tensor_copy(Aib, Ait)
                Aeb = work.tile([32, 128], BF16, tag="Aeb")
                nc.scalar.activation(Aeb, Aet, AFT.Copy)
                Ae2 = work.tile([32, 128], BF16, tag="Ae2")
                nc.vector.tensor_scalar_mul(Ae2, Aet, SQ2)
                at_ps = pspool.tile([128, MT], BF16, tag="ps")
                nc.tensor.transpose(at_ps[:, 0:128], Aib, identb)
                nc.tensor.transpose(at_ps[:, 128:160], Aeb, identb[0:32, 0:32])
                ATb = work.tile([128, MT], BF16, tag="ATb")
                nc.scalar.activation(ATb, at_ps, AFT.Copy)
                yield

                # K = Q + (alpha+sigma) I + rho (Ai'Ai + 2 Ae'Ae); the rho
                # factor is folded into the psum consumer so only unscaled
                # bf16 casts of Ai / sqrt(2) Ae are needed.
                K_ps = pspool.tile([128, 128], F32, tag="ps")
                nc.tensor.matmul(K_ps, Aib, Aib, start=True, stop=False)
                nc.tensor.matmul(K_ps, Ae2, Ae2, start=False, stop=True)
                tmp = work.tile([128, 128], F32, tag="tmp")
                nc.vector.scalar_tensor_tensor(out=tmp, in0=K_ps, scalar=-RHO,
                                               in1=Qt, op0=ALU.mult,
                                               op1=ALU.subtract)
                negK = work.tile([128, 128], F32, tag="negK")
                nc.vector.scalar_tensor_tensor(out=negK, in0=tmp, scalar=1.0,
                                               in1=cIdent, op0=ALU.mult,
                                               op1=ALU.subtract)
                negKb = work.tile([128, 128], BF16, tag="negKb")
                nc.vector.tensor_copy(negKb, negK)
                # NS state lives in bf16 (operand precision); the final fp32
                # polish squares away the bf16 state floor.
                Xb = work.tile([128, 128], BF16, tag="Xb", bufs=8)
                nc.vector.scalar_tensor_tensor(out=Xb, in0=tmp, scalar=C0 * C0,
                                               in1=xinitI, op0=ALU.mult,
                                               op1=ALU.add)
                yield

                for k in range(ns_loop - 2):
                    G1_ps = pspool.tile([128, 128], F32, tag="ps")
                    nc.tensor.matmul(G1_ps, negKb, Xb, start=True, stop=True)
                    g1 = work.tile([128, 128], BF16, tag="g1", bufs=8)
                    nc.scalar.activation(g1, G1_ps, AFT.Copy)
                    X2_ps = pspool.tile([128, 128], F32, tag="ps")
                    nc.tensor.matmul(X2_ps, Xb, g1, start=True, stop=True)
                    Xn = work.tile([128, 128], BF16, tag="Xb", bufs=8)
                    nc.vector.scalar_tensor_tensor(out=Xn, in0=Xb, scalar=2.0,
                                                   in1=X2_ps, op0=ALU.mult,
                                                   op1=ALU.add)
                    Xb = Xn
                    yield

                # fp32 polish iteration: X8 = 2 Xf + Xf negK Xf with Xf the
                # upcast bf16 state; explicit transpose feeds lhsT since Xf is
                # not exactly symmetric.
                XfF = work.tile([128, 128], F32, tag="XfF")
                nc.vector.tensor_copy(XfF, Xb)
                XfT_ps = pspool.tile([128, 128], F32, tag="ps")
                nc.tensor.transpose(XfT_ps, XfF, ident)
                XfT = work.tile([128, 128], F32, tag="XfT")
                nc.vector.tensor_copy(XfT, XfT_ps)
                G1p = pspool.tile([128, 128], F32, tag="ps")
                nc.tensor.matmul(G1p, negK, XfF, start=True, stop=True)
                g1f = work.tile([128, 128], F32, tag="g1f")
                nc.scalar.activation(g1f, G1p, AFT.Copy)
                X2p = pspool.tile([128, 128], F32, tag="ps")
                nc.tensor.matmul(X2p, XfT, g1f, start=True, stop=False)
                nc.tensor.matmul(X2p, twoI, XfF, start=False, stop=True)
                X = work.tile([128, 128], F32, tag="X8")
                nc.vector.tensor_copy(X, X2p)
                Xb8 = work.tile([128, 128], BF16, tag="Xb8")
                nc.vector.tensor_copy(Xb8, X)
                yield

                # M = Kinv At' (bf16 operands; fp32 accumulation in PSUM)
                Ms_ps = pspool.tile([128, MT], F32, tag="ps")
                nc.tensor.matmul(Ms_ps, Xb8, ATb, start=True, stop=True)
                Ms = work.tile([128, MT], F32, tag="Ms")
                nc.vector.tensor_copy(Ms, Ms_ps)
                Msb = work.tile([128, MT], BF16, tag="Msb")
                nc.vector.tensor_copy(Msb, Ms)
                yield

                # H = At Kinv = Ms^T via bf16 PE transposes; spilled bf16.
                # s_vec / d columns go to a small fp32 psum.
                H_ps = pspool.tile([128, 256], BF16, tag="ps")
                nc.tensor.transpose(H_ps[:, 0:128], Msb[:, 0:128], identb)
                nc.tensor.transpose(H_ps[0:32, 128:256], Msb[:, 128:160],
                                    identb)
                sd_ps = pspool.tile([128, 4], F32, tag="ps")
                nc.tensor.matmul(sd_ps[:, 0:1], X, _col(nqv_all, n),
                                 start=True, stop=True, skip_group_check=True)
                nc.tensor.matmul(sd_ps[:, 1:2], Ms[:, 0:128],
                                 _col(nqv_all, n),
                                 start=True, stop=True, skip_group_check=True)
                nc.tensor.matmul(sd_ps[0:32, 2:3], Ms[:, 128:160],
                                 _col(nqv_all, n),
                                 start=True, stop=True, skip_group_check=True)
                Gr1_ps = pspool.tile([128, MT], F32, tag="ps")
                nc.tensor.matmul(Gr1_ps, ATb[:, 0:128], Msb, start=True,
                                 stop=True)
                Gr2_ps = pspool.tile([32, MT], F32, tag="ps")
                nc.tensor.matmul(Gr2_ps, ATb[:, 128:160], Msb, start=True,
                                 stop=True)
                yield

                nc.vector.tensor_copy(_col(S_all, n), sd_ps[:, 0:1])
                nc.vector.tensor_copy(_col(D_all, n), sd_ps[:, 1:2])
                nc.vector.tensor_copy(D_all[0:32, n_el + n:n_el + n + 1],
                                      sd_ps[0:32, 2:3])
                nc.vector.tensor_scalar_mul(t1(n), Gr1_ps[:, 0:128], -1.0)
                nc.vector.tensor_scalar_mul(w4(n), Gr1_ps[:, 128:160], -1.0)
                nc.vector.tensor_scalar_mul(
                    G2A_all[32 * a_:32 * a_ + 32, q_ * 128:(q_ + 1) * 128],
                    Gr2_ps[:, 0:128], -1.0)
                nc.vector.tensor_scalar_mul(g2ed(n), Gr2_ps[:, 128:160], -1.0)
                Htile = work.tile([128, 256], BF16, tag="H")
                nc.scalar.activation(Htile[:, 0:128], H_ps[:, 0:128], AFT.Copy)
                nc.scalar.activation(Htile[0:32, 128:256], H_ps[0:32, 128:256],
                                     AFT.Copy)
                nc.sync.dma_start(out=hsp_d[n, 0:128, :], in_=Htile[:, 0:128])
                nc.sync.dma_start(out=hsp_d[n, 128:160, :],
                                  in_=Htile[0:32, 128:256])

                if taps and n == 0:
                    nc.sync.dma_start(out=dbg_d[0, :, 0:128], in_=negK)
                    nc.sync.dma_start(out=dbg_d[1, :, 0:128], in_=X)
                    nc.sync.dma_start(out=dbg_d[2, :, 0:MT], in_=Ms)
                    nc.sync.dma_start(out=dbg_d[3, :, 0:128], in_=XfF)
                    nc.sync.dma_start(out=dbg_d[4, :, 0:128], in_=g1f)

            K_PIPE = 3
            pend = deque()
            nextn = 0
            while pend or nextn < n_el:
                while len(pend) < K_PIPE and nextn < n_el:
                    pend.append(elem_stages(nextn))
                    nextn += 1
                g = pend.popleft()
                try:
                    next(g)
                    pend.append(g)
                except StopIteration:
                    pass

            # ---------------- s1 init + C' prepass ----------------
            # top: psum = d_i - u_i (s1), then + g0_i (C')
            S1T = pspool.tile([128, n_el], F32, tag="ps")
            nc.tensor.matmul(S1T, negI, u_i, start=True, stop=False,
                             skip_group_check=True)
            nc.tensor.matmul(S1T, ident, D_all[:, 0:n_el], start=False, stop=False,
                             skip_group_check=True)
            nc.vector.tensor_copy(s_i[0], S1T)
            S1E = pspool.tile([32, n_el], F32, tag="ps")
            nc.tensor.matmul(S1E, negI[0:32, 0:32], u_e2, start=True, stop=False,
                             skip_group_check=True)
            nc.tensor.matmul(S1E, ident[0:32, 0:32],
                             D_all[0:32, n_el:2 * n_el], start=False, stop=True,
                             skip_group_check=True)
            nc.vector.tensor_copy(s_e[0][:, 0:n_el], S1E)
            nc.vector.tensor_scalar(out=s_e[0][:, n_el:2 * n_el], in0=S1E,
                                    scalar1=-1.0, scalar2=-EPS_,
                                    op0=ALU.mult, op1=ALU.add)

            # accumulate g0 terms (bf16 G x bf16 -rho*uC) into the psums;
            # the e-block terms go through the quad-packed scratch.
            scr0 = pspool.tile([128, n_el], F32, tag="ps")
            for n in range(n_el):
                nc.tensor.matmul(_col(S1T, n), t1(n), _col(nruC_top, n),
                                 start=False, stop=False, skip_group_check=True)
            for q in range(Q):
                nc.tensor.matmul(S1T[:, 4 * q:4 * q + 4],
                                 G2A_all[:, q * 128:(q + 1) * 128],
                                 nruC_botD[:, 4 * q:4 * q + 4],
                                 start=False, stop=(q == Q - 1),
                                 skip_group_check=True)
            for q in range(Q):
                nc.tensor.matmul(scr0[:, 4 * q:4 * q + 4],
                                 W4_all[:, q * 128:(q + 1) * 128],
                                 nruC_top[:, 4 * q:4 * q + 4],
                                 start=(q == 0), stop=False,
                                 skip_group_check=True)
            for q in range(Q):
                nc.tensor.matmul(scr0[:, 4 * q:4 * q + 4],
                                 G2eD_all[:, q * 128:(q + 1) * 128],
                                 nruC_botD[:, 4 * q:4 * q + 4],
                                 start=False, stop=(q == Q - 1),
                                 skip_group_check=True)
            nc.vector.tensor_copy(Cp_i, S1T)
            for a in range(4):
                nc.vector.tensor_copy(
                    _strided_cols(he_sb[0], a, 4, Q),
                    _strided_cols(scr0, a, 4, Q, part=(32 * a, 32 * a + 32)))
            nc.vector.tensor_tensor(Cp_e[:, 0:n_el], s_e[0][:, 0:n_el],
                                    he_sb[0], ALU.add)
            nc.vector.tensor_scalar(out=Cp_e[:, n_el:2 * n_el],
                                    in0=Cp_e[:, 0:n_el],
                                    scalar1=-1.0, scalar2=-EPS_,
                                    op0=ALU.mult, op1=ALU.add)
            if taps:
                nc.sync.dma_start(out=dbg_d[5, :, 0:n_el], in_=Cp_i)
                nc.sync.dma_start(out=dbg_d[6, :, 0:n_el], in_=s_i[0])

            # ---------------- phase B: ADMM loop ----------------
            # Each half_iter is split into two independent 64-element halves;
            # half h's DVE state-update chain overlaps the other half's PE
            # block, so steady state is pure PE (weight-load bound).  B/Bib/
            # pbot/pbotD for a state index are computed as soon as that state
            # is produced (epilogue of the producing half), so the next PE
            # block never waits on DVE.
            HQ = Q // 2  # quads per half

            def state_tail(dst, h):
                """B / Bib for the i-block of half h of state dst."""
                hs = slice(64 * h, 64 * h + 64)
                nc.scalar.activation(B_i[dst][:, hs], s_i[dst][:, hs],
                                     AFT.Abs, scale=RHO)
                nc.scalar.activation(Bib[dst][:, hs], B_i[dst][:, hs],
                                     AFT.Copy)

            def state_tail_e(dst):
                """B_e / pbot / pbotD for state dst (full width)."""
                nc.scalar.activation(B_e[dst], s_e[dst], AFT.Abs, scale=RHO)
                nc.vector.tensor_tensor(pbot[dst][0:32, :],
                                        B_e[dst][:, 0:n_el],
                                        B_e[dst][:, n_el:2 * n_el],
                                        ALU.subtract)
                nc.vector.tensor_copy(pbot[dst][32:64, :], pbot[dst][0:32, :])
                nc.vector.tensor_copy(pbot[dst][64:128, :], pbot[dst][0:64, :])
                for k in range(4):
                    nc.vector.tensor_copy(
                        _strided_cols(pbotD[dst], k, 4, Q,
                                      part=(32 * k, 32 * k + 32)),
                        _strided_cols(pbot[dst], k, 4, Q,
                                      part=(32 * k, 32 * k + 32)))

            def half_iter(src, dst):
                # e-chain prefix only reads src state: issue up front.
                u1 = wks.tile([32, 2 * n_el], F32, tag="u1")
                nc.vector.scalar_tensor_tensor(out=u1, in0=B_e[src],
                                               scalar=0.5 / RHO, in1=Cp_e,
                                               op0=ALU.mult, op1=ALU.add)
                u2 = wks.tile([32, 2 * n_el], F32, tag="u2")
                nc.vector.scalar_tensor_tensor(out=u2, in0=s_e[src],
                                               scalar=0.5, in1=u1,
                                               op0=ALU.mult, op1=ALU.add)
                banks = []
                for h in (0, 1):
                    hs = slice(64 * h, 64 * h + 64)
                    bankT = pspool.tile([128, 64], F32, tag="ps")
                    scr = pspool.tile([128, 64], F32, tag="ps")
                    banks.append((bankT, scr))
                    for j in range(64):
                        n = 64 * h + j
                        nc.tensor.matmul(_col(bankT, j), t1(n),
                                         _col(Bib[src], n), start=(j == 0),
                                         stop=False, skip_group_check=True)
                    for i in range(HQ):
                        q = HQ * h + i
                        nc.tensor.matmul(bankT[:, 4 * i:4 * i + 4],
                                         G2A_all[:, q * 128:(q + 1) * 128],
                                         pbotD[src][:, 4 * q:4 * q + 4],
                                         start=False, stop=(i == HQ - 1),
                                         skip_group_check=True)
                    for i in range(HQ):
                        q = HQ * h + i
                        nc.tensor.matmul(scr[:, 4 * i:4 * i + 4],
                                         W4_all[:, q * 128:(q + 1) * 128],
                                         Bib[src][:, 4 * q:4 * q + 4],
                                         start=(i == 0), stop=False,
                                         skip_group_check=True)
                    for i in range(HQ):
                        q = HQ * h + i
                        nc.tensor.matmul(scr[:, 4 * i:4 * i + 4],
                                         G2eD_all[:, q * 128:(q + 1) * 128],
                                         pbotD[src][:, 4 * q:4 * q + 4],
                                         start=False, stop=(i == HQ - 1),
                                         skip_group_check=True)
                    # s_i' chain for half h (overlaps the other half's PE)
                    t1x = wks.tile([128, 64], F32, tag=f"t1x{h}")
                    nc.vector.scalar_tensor_tensor(out=t1x,
                                                   in0=B_i[src][:, hs],
                                                   scalar=0.5 / RHO,
                                                   in1=Cp_i[:, hs],
                                                   op0=ALU.mult, op1=ALU.add)
                    t2x = wks.tile([128, 64], F32, tag=f"t2x{h}")
                    nc.vector.scalar_tensor_tensor(out=t2x,
                                                   in0=s_i[src][:, hs],
                                                   scalar=0.5, in1=bankT,
                                                   op0=ALU.mult, op1=ALU.add)
                    nc.vector.tensor_tensor(s_i[dst][:, hs], t1x, t2x,
                                            ALU.add)
                    state_tail(dst, h)
                    # he gather for half h from its scratch psum
                    for a in range(4):
                        nc.vector.tensor_copy(
                            _strided_cols(he_sb[src], 64 * h + a, 4, HQ),
                            _strided_cols(scr, a, 4, HQ,
                                          part=(32 * a, 32 * a + 32)))
                # e-state update (full width) + its tail
                nc.vector.tensor_tensor(s_e[dst][:, 0:n_el],
                                        u2[:, 0:n_el], he_sb[src], ALU.add)
                nc.vector.tensor_tensor(s_e[dst][:, n_el:2 * n_el],
                                        u2[:, n_el:2 * n_el],
                                        he_sb[src], ALU.subtract)
                state_tail_e(dst)

            # prologue: derived state for s1
            state_tail(0, 0)
            state_tail(0, 1)
            state_tail_e(0)
            if n_body > 0:
                with tc.For_i(0, n_body, 1,
                              hint_engines=(mybir.EngineType.PE,)):
                    half_iter(0, 1)
                    half_iter(1, 0)

            # ---------------- final: x = M (rho uC - p~_99) + s_vec -------------
            # B_i[0] / B_e[0] are already maintained by the state tails.
            nc.vector.tensor_tensor(f_bot, B_e[0][:, 0:n_el],
                                    B_e[0][:, n_el:2 * n_el], ALU.subtract)
            nc.vector.tensor_tensor(f_bot, ruC_bot, f_bot, ALU.subtract)
            nc.vector.tensor_tensor(f_top, ruC_top, B_i[0], ALU.subtract)
            ftb = wks.tile([128, n_el], BF16, tag="ftb")
            nc.vector.tensor_copy(ftb, f_top)
            fbb = wks.tile([32, n_el], BF16, tag="fbb")
            nc.vector.tensor_copy(fbb, f_bot)

            xP = pspool.tile([128, n_el], F32, tag="ps")
            nc.tensor.matmul(xP, ident, S_all, start=True, stop=False,
                             skip_group_check=True)
            for n in range(n_el):
                Ht = hre.tile([128, 128], BF16, tag="hret")
                nc.sync.dma_start(out=Ht, in_=hsp_d[n, 0:128, :])
                Hb = hre.tile([32, 128], BF16, tag="hreb")
                nc.sync.dma_start(out=Hb, in_=hsp_d[n, 128:160, :])
                nc.tensor.matmul(_col(xP, n), Ht, _col(ftb, n),
                                 start=False, stop=False, skip_group_check=True)
                nc.tensor.matmul(_col(xP, n), Hb, _col(fbb, n),
                                 start=False, stop=(n == n_el - 1),
                                 skip_group_check=True)
            nc.vector.tensor_copy(xo, xP)
            if taps:
                nc.sync.dma_start(out=dbg_d[7, :, 0:n_el], in_=s_i[0])
            xT = pspool.tile([n_el, 128], F32, tag="ps")
            nc.tensor.transpose(xT, xo, ident)
            nc.vector.tensor_copy(xout, xT)
            nc.sync.dma_start(out=out_d[0:n_el, :, 0], in_=xout)

    nc.compile()
    return nc


_NC_CACHE = {}


def _get_nc(taps=False):
    key = taps
    if key not in _NC_CACHE:
        _NC_CACHE[key] = build(taps=taps)
    return _NC_CACHE[key]


def run(inputs, taps=False, trace=False):
    nc = _get_nc(taps=taps)
    in_maps = []
    for c in range(NCORES):
        sl = slice(c * P, (c + 1) * P)
        in_maps.append({k: np.ascontiguousarray(np.asarray(v)[sl], dtype=np.float32)
                        for k, v in inputs.items()})
    res = run_bass_kernel_spmd(nc, in_maps, core_ids=list(range(NCORES)),
                               trace=trace)
    out = np.concatenate([res.results[c]["out"] for c in range(NCORES)], axis=0)
    return out, res


def kernel(**inputs):
    out, _ = run(inputs)
    return out



# revision 24
# speedup vs baseline: 1.2008x; 1.1356x over previous
"""Trainium2 Bass kernel for nn_BackwardStep_38749194944853.

Batched ADMM QP solve (OSQP-style), N=1024 independent QPs of dim nx=128 with
mi=128 inequality + me=32 doubled equality constraints, 100 fixed iterations.

Strategy (pure data-parallel over 8 cores, 128 QPs per core):
  Phase A (per element, TensorE-heavy):
    K = Q + (1+sigma) I + rho (Ai'Ai + 2 Ae'Ae)
    Kinv via Newton-Schulz (scalar init c*I; K >= 1.1 I by construction)
    M = Kinv At'  (At = [Ai; Ae], 160x128);  G = At M (160x160, symmetric)
    d = At (-Kinv qv)  -> persistent fp32 PSUM bank;  s_vec = -Kinv qv
    Stationary tiles stored bf16: T1 = -G[0:128, 0:160], T2 = -G[128:160, 0:160]
    H = At Kinv (=M^T) spilled to DRAM (fp32) for the final matvec.
  Phase B (98 iterations), state s_t = a_t - u in blocks [i(128); e2(32); e3(32)],
  laid out [m-partitions, element-columns]:
    B   = |rho s|  (fp32 for the exact relu path; bf16 copy feeds the matvec)
    p~  = [B_i ; B_e2-B_e3]
    s' = C' + (0.5/rho) B + 0.5 s - G p~   (+G p~ for the e3 block)
    PSUM accumulates batched identity-stationary matmuls (affine terms, fp32)
    + per-element 4 bf16 matmuls with the stored -G tiles.
  Final: x = M (rho uC - p~_99) + s_vec via the spilled fp32 H as stationary.

Measured on trn2 (8 cores, NTFF profile): 7.27 ms HW exec at the previous
checkpoint, rel err 3.8e-3 vs fp64 reference replica (fp32-everywhere variant:
22.3 ms, rel err 5e-6). Final version additionally moves the per-iteration
affine accumulation off the TensorEngine onto DVE (removes 12 fp32 dual-pass
matmuls per iteration, ~10 us of 97.5 us); HW-verified same rel err.
"""
import os
from collections import deque

import numpy as np

import concourse.bass as bass
import concourse.bacc as bacc
import concourse.mybir as mybir
from concourse.tile import TileContext
from concourse.masks import make_identity
from concourse.bass_utils import run_bass_kernel_spmd

F32 = mybir.dt.float32
BF16 = mybir.dt.bfloat16
ALU = mybir.AluOpType
AFT = mybir.ActivationFunctionType

NCORES = 8
P = 128            # elements per core
NX = 128           # QP dimension
MI = 128           # inequality rows
ME = 32            # equality rows
MT = MI + ME       # 160 collapsed constraint dim

RHO = 0.1
EPS_ = 1e-4
ACOEF = 1.0 + 1e-6          # alpha + sigma added to Q's diagonal
C0 = float(2.0 / (1.1 + 7.5))  # Newton-Schulz scalar init
NS_LOOP = 5                  # NS iterations after the fused first one (6 total)
N_ITER = 100                 # reference ADMM iterations
N_AUPD = N_ITER - 2          # 98 a-state updates (a_1 given, w from a_99)
N_BODY = N_AUPD // 2         # 49 For_i bodies x 2 updates


def _col(t, n):
    return t[:, n:n + 1]


def _strided_cols(t, start, step, count, part=None):
    base = t[:, 0:1] if part is None else t[part[0]:part[1], 0:1]
    return bass.AP(tensor=base.tensor, offset=base.offset + start,
                   ap=[base.ap[0], [step, count]])


def build(n_el=P, n_body=N_BODY, ns_loop=NS_LOOP, taps=False):
    nc = bacc.Bacc()

    x_d = nc.dram_tensor("x", [P, NX, 1], F32, kind="ExternalInput")
    Q_d = nc.dram_tensor("Q", [P, NX, NX], F32, kind="ExternalInput")
    q_d = nc.dram_tensor("q", [P, NX, 1], F32, kind="ExternalInput")
    Ai_d = nc.dram_tensor("A_ineq", [P, MI, NX], F32, kind="ExternalInput")
    bi_d = nc.dram_tensor("b_ineq", [P, MI, 1], F32, kind="ExternalInput")
    Ae_d = nc.dram_tensor("A_eq", [P, ME, NX], F32, kind="ExternalInput")
    be_d = nc.dram_tensor("b_eq", [P, ME, 1], F32, kind="ExternalInput")
    out_d = nc.dram_tensor("out", [P, NX, 1], F32, kind="ExternalOutput")
    hsp_d = nc.dram_tensor("hspill", [P, 128, 256], BF16)  # internal DRAM
    if taps:
        dbg_d = nc.dram_tensor("dbg", [8, 128, 256], F32, kind="ExternalOutput")

    with TileContext(nc) as tc:
        with (
            tc.tile_pool(name="consts", bufs=1) as consts,
            tc.tile_pool(name="gpool", bufs=1) as gpool,
            tc.tile_pool(name="work", bufs=4) as work,
            tc.tile_pool(name="wks", bufs=2) as wks,
            tc.tile_pool(name="hre", bufs=8) as hre,
            tc.tile_pool(name="pspool", bufs=8, space="PSUM") as pspool,
        ):
            # ---------------- constants ----------------
            ident = consts.tile([128, 128], F32)
            make_identity(nc, ident)
            negI = consts.tile([128, 128], F32)
            nc.vector.tensor_scalar_mul(negI, ident, -1.0)
            halfI = consts.tile([128, 128], F32)
            nc.vector.tensor_scalar_mul(halfI, ident, 0.5)
            hbrI = consts.tile([128, 128], F32)
            nc.vector.tensor_scalar_mul(hbrI, ident, 0.5 / RHO)
            twoI = consts.tile([128, 128], F32)
            nc.vector.tensor_scalar_mul(twoI, ident, 2.0)
            twoCid = consts.tile([128, 128], F32)
            nc.vector.tensor_scalar_mul(twoCid, ident, 2.0 * C0)
            cIdent = consts.tile([128, 128], F32)
            nc.vector.tensor_scalar_mul(cIdent, ident, ACOEF)
            xinitI = consts.tile([128, 128], F32)
            nc.vector.tensor_scalar_mul(xinitI, ident, 2.0 * C0 - C0 * C0 * ACOEF)
            twoIb = consts.tile([128, 128], BF16)
            nc.vector.tensor_scalar_mul(twoIb, ident, 2.0)
            identb = consts.tile([128, 128], BF16)
            nc.vector.tensor_copy(identb, ident)

            # ---------------- persistent big tiles ----------------
            Q = n_el // 4  # quads: element n = 4q+k at partition block 32k
            # T1_all: per element -G[0:128, 0:128] bf16, [128, n_el*128]
            T1_all = gpool.tile([128, n_el * 128], BF16)
            # G2A_all: quad-stacked -G[128:160, 0:128] chunks: element 4q+k at
            # partitions 32k, cols q*128..; used as one [128,128] lhsT per quad
            # with a block-sparse rhs.
            G2A_all = gpool.tile([128, Q * 128], BF16)
            # W4_all: quad-packed -G[0:128, 128:160] blocks: element 4q+k in
            # cols q*128+32k.. (M-packed); one [128,128] FWL lhsT per quad,
            # dense rhs cols, element k's result in psum rows 32k.
            W4_all = gpool.tile([128, Q * 128], BF16)
            # G2eD_all: quad block-diagonal -G[128:160, 128:160] blocks:
            # element 4q+k at rows 32k, cols q*128+32k..; zeros elsewhere, so
            # a block-sparse rhs accumulates garbage-free onto W4's rows.
            G2eD_all = gpool.tile([128, Q * 128], BF16)

            def t1(n):
                return T1_all[:, n * 128:(n + 1) * 128]

            def w4(n):
                a, g = n % 4, n // 4
                return W4_all[:, g * 128 + 32 * a:g * 128 + 32 * a + 32]

            def g2ed(n):
                a, g = n % 4, n // 4
                return G2eD_all[32 * a:32 * a + 32,
                                g * 128 + 32 * a:g * 128 + 32 * a + 32]

            # batched constants (m-layout: [m-part, element-cols])
            u_i = gpool.tile([128, n_el], F32)
            be_t = gpool.tile([32, n_el], F32)
            u_e2 = gpool.tile([32, n_el], F32)
            ruC_top = gpool.tile([128, n_el], F32)
            ruC_bot = gpool.tile([32, n_el], F32)
            nruC_top = gpool.tile([128, n_el], BF16)
            nruC_bot = gpool.tile([128, n_el], BF16)   # replicated x4
            nruC_botD = gpool.tile([128, n_el], BF16)  # block-sparse diag scatter
            nqv_all = gpool.tile([128, n_el], F32)
            Cp_i = gpool.tile([128, n_el], F32)
            Cp_e = gpool.tile([32, 2 * n_el], F32)     # [Cp_e2 | Cp_e3]
            S_all = gpool.tile([128, n_el], F32)
            D_all = gpool.tile([128, 2 * n_el], F32)   # [d_top | d_bot(32p)]
            # ADMM state (ping-pong a/b)
            s_i = [gpool.tile([128, n_el], F32, name=f"s_i{j}") for j in range(2)]
            s_e = [gpool.tile([32, 2 * n_el], F32, name=f"s_e{j}") for j in range(2)]
            B_i = [gpool.tile([128, n_el], F32, name=f"B_i{j}") for j in range(2)]
            B_e = [gpool.tile([32, 2 * n_el], F32, name=f"B_e{j}") for j in range(2)]
            Bib = [gpool.tile([128, n_el], BF16, name=f"Bib{j}") for j in range(2)]
            pbot = [gpool.tile([128, n_el], BF16, name=f"pbot{j}") for j in range(2)]
            pbotD = [gpool.tile([128, n_el], BF16, name=f"pbotD{j}") for j in range(2)]
            he_sb = [gpool.tile([32, n_el], F32, name=f"he_sb{j}") for j in range(2)]
            f_top = gpool.tile([128, n_el], F32)
            f_bot = gpool.tile([32, n_el], F32)
            xo = gpool.tile([128, n_el], F32)
            xout = gpool.tile([n_el, 128], F32)

            nc.vector.memset(pbotD[0], 0.0)
            nc.vector.memset(pbotD[1], 0.0)
            nc.vector.memset(nruC_botD, 0.0)
            nc.vector.memset(G2eD_all, 0.0)

            # ---------------- batched input prep ----------------
            x_el = wks.tile([P, NX], F32, tag="xel")
            q_el = wks.tile([P, NX], F32, tag="qel")
            nc.sync.dma_start(out=x_el, in_=x_d[:, :, 0])
            nc.sync.dma_start(out=q_el, in_=q_d[:, :, 0])
            nq_el = wks.tile([P, NX], F32, tag="nqel")
            nc.vector.tensor_tensor(nq_el, x_el, q_el, ALU.subtract)  # -(q - x)
            nqps = pspool.tile([128, P], F32, tag="ps")
            nc.tensor.transpose(nqps, nq_el, ident)
            nc.vector.tensor_copy(nqv_all, nqps[:, 0:n_el])

            bi_el = wks.tile([P, MI], F32, tag="biel")
            nc.sync.dma_start(out=bi_el, in_=bi_d[:, :, 0])
            bips = pspool.tile([128, P], F32, tag="ps")
            nc.tensor.transpose(bips, bi_el, ident)
            nc.vector.tensor_copy(u_i, bips[:, 0:n_el])

            be_el = wks.tile([P, ME], F32, tag="beel")
            nc.sync.dma_start(out=be_el, in_=be_d[:, :, 0])
            beps = pspool.tile([32, P], F32, tag="ps")
            nc.tensor.transpose(beps, be_el, ident)
            nc.vector.tensor_copy(be_t, beps[:, 0:n_el])

            nc.vector.tensor_scalar_add(u_e2, be_t, EPS_)
            nc.vector.tensor_scalar_mul(ruC_top, u_i, RHO)
            nc.vector.tensor_scalar(out=ruC_bot, in0=be_t, scalar1=2.0 * RHO,
                                    scalar2=RHO * EPS_, op0=ALU.mult, op1=ALU.add)
            nc.vector.tensor_scalar_mul(nruC_top, u_i, -RHO)
            nc.vector.tensor_scalar(out=nruC_bot[0:32, :], in0=be_t,
                                    scalar1=-2.0 * RHO, scalar2=-RHO * EPS_,
                                    op0=ALU.mult, op1=ALU.add)
            nc.vector.tensor_copy(nruC_bot[32:64, :], nruC_bot[0:32, :])
            nc.vector.tensor_copy(nruC_bot[64:128, :], nruC_bot[0:64, :])
            for k in range(4):
                nc.vector.tensor_copy(
                    _strided_cols(nruC_botD, k, 4, Q, part=(32 * k, 32 * k + 32)),
                    _strided_cols(nruC_bot, k, 4, Q, part=(32 * k, 32 * k + 32)))

            # ---------------- phase A: per-element factorization ----------------
            # Emitted as a K_PIPE-way software pipeline: each element's chain
            # is a staged generator and stages of neighbouring elements are
            # interleaved in issue order, so the strict-FIFO ACT/DVE queues
            # overlap work across elements instead of head-of-line blocking.
            SQ2 = float(np.sqrt(2.0))

            def elem_stages(n):
                a_, q_ = n % 4, n // 4
                Qt = work.tile([128, 128], F32, tag="Q")
                nc.sync.dma_start(out=Qt, in_=Q_d[n])
                Ait = work.tile([128, 128], F32, tag="Ai")
                nc.sync.dma_start(out=Ait, in_=Ai_d[n])
                Aet = work.tile([32, 128], F32, tag="Ae")
                nc.sync.dma_start(out=Aet, in_=Ae_d[n])
                yield

                Aib = work.tile([128, 128], BF16, tag="Aib")
                nc.vector.tensor_copy(Aib, Ait)
                Aeb = work.tile([32, 128], BF16, tag="Aeb")
                nc.scalar.activation(Aeb, Aet, AFT.Copy)
                Ae2 = work.tile([32, 128], BF16, tag="Ae2")
                nc.vector.tensor_scalar_mul(Ae2, Aet, SQ2)
                at_ps = pspool.tile([128, MT], BF16, tag="ps")
                nc.tensor.transpose(at_ps[:, 0:128], Aib, identb)
                nc.tensor.transpose(at_ps[:, 128:160], Aeb, identb[0:32, 0:32])
                ATb = work.tile([128, MT], BF16, tag="ATb")
                nc.scalar.activation(ATb, at_ps, AFT.Copy)
                yield

                # K = Q + (alpha+sigma) I + rho (Ai'Ai + 2 Ae'Ae); the rho
                # factor is folded into the psum consumer so only unscaled
                # bf16 casts of Ai / sqrt(2) Ae are needed.
                K_ps = pspool.tile([128, 128], F32, tag="ps")
                nc.tensor.matmul(K_ps, Aib, Aib, start=True, stop=False)
                nc.tensor.matmul(K_ps, Ae2, Ae2, start=False, stop=True)
                tmp = work.tile([128, 128], F32, tag="tmp")
                nc.vector.scalar_tensor_tensor(out=tmp, in0=K_ps, scalar=-RHO,
                                               in1=Qt, op0=ALU.mult,
                                               op1=ALU.subtract)
                negK = work.tile([128, 128], F32, tag="negK")
                nc.vector.scalar_tensor_tensor(out=negK, in0=tmp, scalar=1.0,
                                               in1=cIdent, op0=ALU.mult,
                                               op1=ALU.subtract)
                negKb = work.tile([128, 128], BF16, tag="negKb")
                nc.vector.tensor_copy(negKb, negK)
                # NS state lives in bf16 (operand precision); the final fp32
                # polish squares away the bf16 state floor.
                Xb = work.tile([128, 128], BF16, tag="Xb", bufs=8)
                nc.vector.scalar_tensor_tensor(out=Xb, in0=tmp, scalar=C0 * C0,
                                               in1=xinitI, op0=ALU.mult,
                                               op1=ALU.add)
                yield

                for k in range(ns_loop - 2):
                    G1_ps = pspool.tile([128, 128], F32, tag="ps")
                    nc.tensor.matmul(G1_ps, negKb, Xb, start=True, stop=True)
                    g1 = work.tile([128, 128], BF16, tag="g1", bufs=8)
                    nc.scalar.activation(g1, G1_ps, AFT.Copy)
                    X2_ps = pspool.tile([128, 128], F32, tag="ps")
                    nc.tensor.matmul(X2_ps, Xb, g1, start=True, stop=True)
                    Xn = work.tile([128, 128], BF16, tag="Xb", bufs=8)
                    nc.vector.scalar_tensor_tensor(out=Xn, in0=Xb, scalar=2.0,
                                                   in1=X2_ps, op0=ALU.mult,
                                                   op1=ALU.add)
                    Xb = Xn
                    yield

                # fp32 polish iteration: X8 = 2 Xf + Xf negK Xf with Xf the
                # upcast bf16 state; explicit transpose feeds lhsT since Xf is
                # not exactly symmetric.
                XfF = work.tile([128, 128], F32, tag="XfF")
                nc.vector.tensor_copy(XfF, Xb)
                XfT_ps = pspool.tile([128, 128], F32, tag="ps")
                nc.tensor.transpose(XfT_ps, XfF, ident)
                XfT = work.tile([128, 128], F32, tag="XfT")
                nc.vector.tensor_copy(XfT, XfT_ps)
                G1p = pspool.tile([128, 128], F32, tag="ps")
                nc.tensor.matmul(G1p, negK, XfF, start=True, stop=True)
                g1f = work.tile([128, 128], F32, tag="g1f")
                nc.scalar.activation(g1f, G1p, AFT.Copy)
                X2p = pspool.tile([128, 128], F32, tag="ps")
                nc.tensor.matmul(X2p, XfT, g1f, start=True, stop=False)
                nc.tensor.matmul(X2p, twoI, XfF, start=False, stop=True)
                X = work.tile([128, 128], F32, tag="X8")
                nc.vector.tensor_copy(X, X2p)
                Xb8 = work.tile([128, 128], BF16, tag="Xb8")
                nc.vector.tensor_copy(Xb8, X)
                yield

                # M = Kinv At' (bf16 operands; fp32 accumulation in PSUM)
                Ms_ps = pspool.tile([128, MT], F32, tag="ps")
                nc.tensor.matmul(Ms_ps, Xb8, ATb, start=True, stop=True)
                Ms = work.tile([128, MT], F32, tag="Ms")
                nc.vector.tensor_copy(Ms, Ms_ps)
                Msb = work.tile([128, MT], BF16, tag="Msb")
                nc.vector.tensor_copy(Msb, Ms)
                yield

                # H = At Kinv = Ms^T via bf16 PE transposes; spilled bf16.
                # s_vec / d columns go to a small fp32 psum.
                H_ps = pspool.tile([128, 256], BF16, tag="ps")
                nc.tensor.transpose(H_ps[:, 0:128], Msb[:, 0:128], identb)
                nc.tensor.transpose(H_ps[0:32, 128:256], Msb[:, 128:160],
                                    identb)
                sd_ps = pspool.tile([128, 4], F32, tag="ps")
                nc.tensor.matmul(sd_ps[:, 0:1], X, _col(nqv_all, n),
                                 start=True, stop=True, skip_group_check=True)
                nc.tensor.matmul(sd_ps[:, 1:2], Ms[:, 0:128],
                                 _col(nqv_all, n),
                                 start=True, stop=True, skip_group_check=True)
                nc.tensor.matmul(sd_ps[0:32, 2:3], Ms[:, 128:160],
                                 _col(nqv_all, n),
                                 start=True, stop=True, skip_group_check=True)
                Gr1_ps = pspool.tile([128, MT], F32, tag="ps")
                nc.tensor.matmul(Gr1_ps, ATb[:, 0:128], Msb, start=True,
                                 stop=True)
                Gr2_ps = pspool.tile([32, MT], F32, tag="ps")
                nc.tensor.matmul(Gr2_ps, ATb[:, 128:160], Msb, start=True,
                                 stop=True)
                yield

                nc.vector.tensor_copy(_col(S_all, n), sd_ps[:, 0:1])
                nc.vector.tensor_copy(_col(D_all, n), sd_ps[:, 1:2])
                nc.vector.tensor_copy(D_all[0:32, n_el + n:n_el + n + 1],
                                      sd_ps[0:32, 2:3])
                nc.vector.tensor_scalar_mul(t1(n), Gr1_ps[:, 0:128], -1.0)
                nc.vector.tensor_scalar_mul(w4(n), Gr1_ps[:, 128:160], -1.0)
                nc.vector.tensor_scalar_mul(
                    G2A_all[32 * a_:32 * a_ + 32, q_ * 128:(q_ + 1) * 128],
                    Gr2_ps[:, 0:128], -1.0)
                nc.vector.tensor_scalar_mul(g2ed(n), Gr2_ps[:, 128:160], -1.0)
                Htile = work.tile([128, 256], BF16, tag="H")
                nc.scalar.activation(Htile[:, 0:128], H_ps[:, 0:128], AFT.Copy)
                nc.scalar.activation(Htile[0:32, 128:256], H_ps[0:32, 128:256],
                                     AFT.Copy)
                nc.sync.dma_start(out=hsp_d[n], in_=Htile)

                if taps and n == 0:
                    nc.sync.dma_start(out=dbg_d[0, :, 0:128], in_=negK)
                    nc.sync.dma_start(out=dbg_d[1, :, 0:128], in_=X)
                    nc.sync.dma_start(out=dbg_d[2, :, 0:MT], in_=Ms)
                    nc.sync.dma_start(out=dbg_d[3, :, 0:128], in_=XfF)
                    nc.sync.dma_start(out=dbg_d[4, :, 0:128], in_=g1f)

            K_PIPE = 4
            pend = deque()
            nextn = 0
            while pend or nextn < n_el:
                while len(pend) < K_PIPE and nextn < n_el:
                    pend.append(elem_stages(nextn))
                    nextn += 1
                g = pend.popleft()
                try:
                    next(g)
                    pend.append(g)
                except StopIteration:
                    pass

            # ---------------- s1 init + C' prepass ----------------
            # top: psum = d_i - u_i (s1), then + g0_i (C')
            S1T = pspool.tile([128, n_el], F32, tag="ps")
            nc.tensor.matmul(S1T, negI, u_i, start=True, stop=False,
                             skip_group_check=True)
            nc.tensor.matmul(S1T, ident, D_all[:, 0:n_el], start=False, stop=False,
                             skip_group_check=True)
            nc.vector.tensor_copy(s_i[0], S1T)
            S1E = pspool.tile([32, n_el], F32, tag="ps")
            nc.tensor.matmul(S1E, negI[0:32, 0:32], u_e2, start=True, stop=False,
                             skip_group_check=True)
            nc.tensor.matmul(S1E, ident[0:32, 0:32],
                             D_all[0:32, n_el:2 * n_el], start=False, stop=True,
                             skip_group_check=True)
            nc.vector.tensor_copy(s_e[0][:, 0:n_el], S1E)
            nc.vector.tensor_scalar(out=s_e[0][:, n_el:2 * n_el], in0=S1E,
                                    scalar1=-1.0, scalar2=-EPS_,
                                    op0=ALU.mult, op1=ALU.add)

            # accumulate g0 terms (bf16 G x bf16 -rho*uC) into the psums;
            # the e-block terms go through the quad-packed scratch.
            scr0 = pspool.tile([128, n_el], F32, tag="ps")
            for n in range(n_el):
                nc.tensor.matmul(_col(S1T, n), t1(n), _col(nruC_top, n),
                                 start=False, stop=False, skip_group_check=True)
            for q in range(Q):
                nc.tensor.matmul(S1T[:, 4 * q:4 * q + 4],
                                 G2A_all[:, q * 128:(q + 1) * 128],
                                 nruC_botD[:, 4 * q:4 * q + 4],
                                 start=False, stop=(q == Q - 1),
                                 skip_group_check=True)
            for q in range(Q):
                nc.tensor.matmul(scr0[:, 4 * q:4 * q + 4],
                                 W4_all[:, q * 128:(q + 1) * 128],
                                 nruC_top[:, 4 * q:4 * q + 4],
                                 start=(q == 0), stop=False,
                                 skip_group_check=True)
            for q in range(Q):
                nc.tensor.matmul(scr0[:, 4 * q:4 * q + 4],
                                 G2eD_all[:, q * 128:(q + 1) * 128],
                                 nruC_botD[:, 4 * q:4 * q + 4],
                                 start=False, stop=(q == Q - 1),
                                 skip_group_check=True)
            nc.vector.tensor_copy(Cp_i, S1T)
            for a in range(4):
                nc.vector.tensor_copy(
                    _strided_cols(he_sb[0], a, 4, Q),
                    _strided_cols(scr0, a, 4, Q, part=(32 * a, 32 * a + 32)))
            nc.vector.tensor_tensor(Cp_e[:, 0:n_el], s_e[0][:, 0:n_el],
                                    he_sb[0], ALU.add)
            nc.vector.tensor_scalar(out=Cp_e[:, n_el:2 * n_el],
                                    in0=Cp_e[:, 0:n_el],
                                    scalar1=-1.0, scalar2=-EPS_,
                                    op0=ALU.mult, op1=ALU.add)
            if taps:
                nc.sync.dma_start(out=dbg_d[5, :, 0:n_el], in_=Cp_i)
                nc.sync.dma_start(out=dbg_d[6, :, 0:n_el], in_=s_i[0])

            # ---------------- phase B: ADMM loop ----------------
            # Each half_iter is split into two independent 64-element halves;
            # half h's DVE state-update chain overlaps the other half's PE
            # block, so steady state is pure PE (weight-load bound).  B/Bib/
            # pbot/pbotD for a state index are computed as soon as that state
            # is produced (epilogue of the producing half), so the next PE
            # block never waits on DVE.
            HQ = Q // 2  # quads per half

            def state_tail(dst, h):
                """B / Bib for the i-block of half h of state dst."""
                hs = slice(64 * h, 64 * h + 64)
                nc.scalar.activation(B_i[dst][:, hs], s_i[dst][:, hs],
                                     AFT.Abs, scale=RHO)
                nc.scalar.activation(Bib[dst][:, hs], B_i[dst][:, hs],
                                     AFT.Copy)

            def state_tail_e(dst):
                """B_e / pbot / pbotD for state dst (full width)."""
                nc.scalar.activation(B_e[dst], s_e[dst], AFT.Abs, scale=RHO)
                nc.vector.tensor_tensor(pbot[dst][0:32, :],
                                        B_e[dst][:, 0:n_el],
                                        B_e[dst][:, n_el:2 * n_el],
                                        ALU.subtract)
                nc.vector.tensor_copy(pbot[dst][32:64, :], pbot[dst][0:32, :])
                nc.vector.tensor_copy(pbot[dst][64:128, :], pbot[dst][0:64, :])
                for k in range(4):
                    nc.vector.tensor_copy(
                        _strided_cols(pbotD[dst], k, 4, Q,
                                      part=(32 * k, 32 * k + 32)),
                        _strided_cols(pbot[dst], k, 4, Q,
                                      part=(32 * k, 32 * k + 32)))

            def half_iter(src, dst):
                # e-chain prefix only reads src state: issue up front.
                u1 = wks.tile([32, 2 * n_el], F32, tag="u1")
                nc.vector.scalar_tensor_tensor(out=u1, in0=B_e[src],
                                               scalar=0.5 / RHO, in1=Cp_e,
                                               op0=ALU.mult, op1=ALU.add)
                u2 = wks.tile([32, 2 * n_el], F32, tag="u2")
                nc.vector.scalar_tensor_tensor(out=u2, in0=s_e[src],
                                               scalar=0.5, in1=u1,
                                               op0=ALU.mult, op1=ALU.add)
                banks = []
                for h in (0, 1):
                    hs = slice(64 * h, 64 * h + 64)
                    bankT = pspool.tile([128, 64], F32, tag="ps")
                    scr = pspool.tile([128, 64], F32, tag="ps")
                    banks.append((bankT, scr))
                    for j in range(64):
                        n = 64 * h + j
                        nc.tensor.matmul(_col(bankT, j), t1(n),
                                         _col(Bib[src], n), start=(j == 0),
                                         stop=False, skip_group_check=True)
                    for i in range(HQ):
                        q = HQ * h + i
                        nc.tensor.matmul(bankT[:, 4 * i:4 * i + 4],
                                         G2A_all[:, q * 128:(q + 1) * 128],
                                         pbotD[src][:, 4 * q:4 * q + 4],
                                         start=False, stop=(i == HQ - 1),
                                         skip_group_check=True)
                    for i in range(HQ):
                        q = HQ * h + i
                        nc.tensor.matmul(scr[:, 4 * i:4 * i + 4],
                                         W4_all[:, q * 128:(q + 1) * 128],
                                         Bib[src][:, 4 * q:4 * q + 4],
                                         start=(i == 0), stop=False,
                                         skip_group_check=True)
                    for i in range(HQ):
                        q = HQ * h + i
                        nc.tensor.matmul(scr[:, 4 * i:4 * i + 4],
                                         G2eD_all[:, q * 128:(q + 1) * 128],
                                         pbotD[src][:, 4 * q:4 * q + 4],
                                         start=False, stop=(i == HQ - 1),
                                         skip_group_check=True)
                    # s_i' chain for half h (overlaps the other half's PE)
                    t1x = wks.tile([128, 64], F32, tag=f"t1x{h}")
                    nc.vector.scalar_tensor_tensor(out=t1x,
                                                   in0=B_i[src][:, hs],
                                                   scalar=0.5 / RHO,
                                                   in1=Cp_i[:, hs],
                                                   op0=ALU.mult, op1=ALU.add)
                    t2x = wks.tile([128, 64], F32, tag=f"t2x{h}")
                    nc.vector.scalar_tensor_tensor(out=t2x,
                                                   in0=s_i[src][:, hs],
                                                   scalar=0.5, in1=bankT,
                                                   op0=ALU.mult, op1=ALU.add)
                    nc.vector.tensor_tensor(s_i[dst][:, hs], t1x, t2x,
                                            ALU.add)
                    state_tail(dst, h)
                    # he gather for half h from its scratch psum
                    for a in range(4):
                        nc.vector.tensor_copy(
                            _strided_cols(he_sb[src], 64 * h + a, 4, HQ),
                            _strided_cols(scr, a, 4, HQ,
                                          part=(32 * a, 32 * a + 32)))
                # e-state update (full width) + its tail
                nc.vector.tensor_tensor(s_e[dst][:, 0:n_el],
                                        u2[:, 0:n_el], he_sb[src], ALU.add)
                nc.vector.tensor_tensor(s_e[dst][:, n_el:2 * n_el],
                                        u2[:, n_el:2 * n_el],
                                        he_sb[src], ALU.subtract)
                state_tail_e(dst)

            # prologue: derived state for s1
            state_tail(0, 0)
            state_tail(0, 1)
            state_tail_e(0)
            if n_body > 0:
                with tc.For_i(0, n_body, 1,
                              hint_engines=(mybir.EngineType.PE,)):
                    half_iter(0, 1)
                    half_iter(1, 0)

            # ---------------- final: x = M (rho uC - p~_99) + s_vec -------------
            # B_i[0] / B_e[0] are already maintained by the state tails.
            nc.vector.tensor_tensor(f_bot, B_e[0][:, 0:n_el],
                                    B_e[0][:, n_el:2 * n_el], ALU.subtract)
            nc.vector.tensor_tensor(f_bot, ruC_bot, f_bot, ALU.subtract)
            nc.vector.tensor_tensor(f_top, ruC_top, B_i[0], ALU.subtract)
            ftb = wks.tile([128, n_el], BF16, tag="ftb")
            nc.vector.tensor_copy(ftb, f_top)
            fbb = wks.tile([32, n_el], BF16, tag="fbb")
            nc.vector.tensor_copy(fbb, f_bot)

            xP = pspool.tile([128, n_el], F32, tag="ps")
            nc.tensor.matmul(xP, ident, S_all, start=True, stop=False,
                             skip_group_check=True)
            for n in range(n_el):
                Hfull = hre.tile([128, 256], BF16, tag="hret", bufs=12)
                nc.sync.dma_start(out=Hfull, in_=hsp_d[n])
                nc.tensor.matmul(_col(xP, n), Hfull[:, 0:128], _col(ftb, n),
                                 start=False, stop=False, skip_group_check=True)
                nc.tensor.matmul(_col(xP, n), Hfull[0:32, 128:256],
                                 _col(fbb, n),
                                 start=False, stop=(n == n_el - 1),
                                 skip_group_check=True)
            nc.vector.tensor_copy(xo, xP)
            if taps:
                nc.sync.dma_start(out=dbg_d[7, :, 0:n_el], in_=s_i[0])
            xT = pspool.tile([n_el, 128], F32, tag="ps")
            nc.tensor.transpose(xT, xo, ident)
            nc.vector.tensor_copy(xout, xT)
            nc.sync.dma_start(out=out_d[0:n_el, :, 0], in_=xout)

    nc.compile()
    return nc


_NC_CACHE = {}


def _get_nc(taps=False):
    key = taps
    if key not in _NC_CACHE:
        _NC_CACHE[key] = build(taps=taps)
    return _NC_CACHE[key]


def run(inputs, taps=False, trace=False):
    nc = _get_nc(taps=taps)
    in_maps = []
    for c in range(NCORES):
        sl = slice(c * P, (c + 1) * P)
        in_maps.append({k: np.ascontiguousarray(np.asarray(v)[sl], dtype=np.float32)
                        for k, v in inputs.items()})
    res = run_bass_kernel_spmd(nc, in_maps, core_ids=list(range(NCORES)),
                               trace=trace)
    out = np.concatenate([res.results[c]["out"] for c in range(NCORES)], axis=0)
    return out, res


def kernel(**inputs):
    out, _ = run(inputs)
    return out

